# revision 7
# baseline (speedup 1.0000x reference)
"""Trainium2 Bass kernel for the Enigma-style CopyMemoryModel.

Math (validated vs reference, see check_fold.py):
  - The lax.scan carries nothing -> every timestep t is independent.
  - t < 128 and d = 1024  =>  rotors 1,2 have pos = 0 (no roll); only rotor 0
    rolls by t, and roll(roll(h,-t) @ W, t) == h @ roll(W, (t,t), (0,1)).
  - Everything before the first rev block is LINEAR with the only t-dependence
    being rotor 0's roll -> fold on host into per-t head matrices
        Mhead_t = [P@Wi | P@bi].T @ roll(rotW0,(t,t)) @ (rotW1@rotW2)   [65,1024]
  - Everything after the last rev block is linear too -> per-t tail matrices
        Mtail_t = (rotW1@rotW2).T @ roll(rotW0,(t,t)).T @ (P@Wo.T)      [1024,64]
  - On chip only:  head (per-t small matmuls), 6 rev couplings fwd,
    Srefl = R+R.T big stage, 6 rev couplings bwd, tail.  bo added on host.
  - Layout on chip: activations stored transposed, hT[128 part, 8 blocks x 1024
    tokens] per core; every stage is out_block[jt] = sum_kt W[kt,jt].T @ h[kt].
  - bf16 datapath (PSUM accumulation fp32, final output fp32): bf16 weights
    enable background LDWEIGHTS + fast-weight-load, so N=512 matmuls stream at
    ~216 ns instead of fp32r's serialized ~355 ns.

Sharding: time-sharded; core c handles t in [c*16, (c+1)*16), all 64 batch
samples -> 1024 tokens per core, token column = g*64 + b.
"""
import numpy as np

B, S, DIN, D, DOUT = 64, 128, 64, 1024, 64
NCORES = 8
TLOC = S // NCORES          # 16 timesteps per core
NTOK = B * TLOC             # 1024 tokens per core
NB = D // 128               # 8 row blocks
NCH = NTOK // 512           # 2 column chunks of 512
HB = 512                    # half of D (rev-block split)
NBH = HB // 128             # 4 blocks per half

_compiled = {}


def _build():
    import concourse.bacc as bacc
    import concourse.mybir as mybir
    from concourse.tile import TileContext

    f32 = mybir.dt.float32
    bf16 = mybir.dt.bfloat16
    ACT_TANH = mybir.ActivationFunctionType.Tanh
    ACT_COPY = mybir.ActivationFunctionType.Copy

    nc = bacc.Bacc(None, target_bir_lowering=False, debug=True)

    xt_d = nc.dram_tensor("xt", [DIN + 1, NTOK], bf16, kind="ExternalInput")
    mh_d = nc.dram_tensor("mhead", [DIN + 1, TLOC * D], bf16, kind="ExternalInput")
    wf_d = nc.dram_tensor("wf", [3, 128, NBH * HB], bf16, kind="ExternalInput")
    wg_d = nc.dram_tensor("wg", [3, 128, NBH * HB], bf16, kind="ExternalInput")
    ws_d = nc.dram_tensor("wsrefl", [128, NB * D], bf16, kind="ExternalInput")
    mt_d = nc.dram_tensor("mtail", [128, TLOC * HB], bf16, kind="ExternalInput")
    out_d = nc.dram_tensor("out", [DOUT, NTOK], f32, kind="ExternalOutput")

    with TileContext(nc) as tc:
        with (
            tc.tile_pool(name="hbuf", bufs=1) as hpool,
            tc.tile_pool(name="wpool", bufs=2) as wpool,
            tc.tile_pool(name="fgpool", bufs=1) as fgpool,
            tc.tile_pool(name="hdpool", bufs=1) as hdpool,
            tc.tile_pool(name="cpool", bufs=1) as cpool,
            tc.tile_pool(name="tpool", bufs=2) as tpool,
            tc.tile_pool(name="ps1", bufs=6, space="PSUM") as ps1,
            tc.tile_pool(name="psw", bufs=2, space="PSUM") as psw,
        ):
            hA = hpool.tile([128, NB * NTOK], bf16)
            hB = hpool.tile([128, NB * NTOK], bf16)
            hAR = hA[:].rearrange("p (n t) -> p n t", n=NB)

            xt = cpool.tile([DIN + 1, NTOK], bf16)
            outsb = cpool.tile([DOUT, NTOK], f32)

            # PE-warmup matmuls on a memset tile queue first so the tensor
            # engine has work while the input DMAs land.
            junk = cpool.tile([128, 512], bf16)
            nc.gpsimd.memset(junk[:], 0.0)

            def junk_mm(n):
                for r in range(n):
                    wps = psw.tile([128, 512], f32, tag="sm")
                    nc.tensor.matmul(wps[:, 0:128], junk[:, 0:128],
                                     junk[:, 0:128], start=True, stop=True)

            with nc.named_scope("warmup"):
                junk_mm(12)

            nc.sync.dma_start(xt[:], xt_d[:])

            # head matrices in 4 batched DMAs (dma_start issue costs ~0.75us
            # each on the sync queue; 16 small ones starve the head)
            mh_tiles = []
            for q in range(4):
                mh = hdpool.tile([DIN + 1, 4 * D], bf16, tag=f"mh{q}",
                                 name=f"mh{q}")
                nc.sync.dma_start(mh[:], mh_d[:, q * 4 * D:(q + 1) * 4 * D])
                mh_tiles.append(mh)

            wfT = [fgpool.tile([128, NBH * HB], bf16, tag=f"wf{i}",
                               name=f"wf{i}") for i in range(3)]
            wgT = [fgpool.tile([128, NBH * HB], bf16, tag=f"wg{i}",
                               name=f"wg{i}") for i in range(3)]
            nc.sync.dma_start(wfT[0][:], wf_d[0])
            nc.sync.dma_start(wgT[0][:], wg_d[0])
            for i in range(1, 3):
                nc.sync.dma_start(wfT[i][:], wf_d[i])
                nc.sync.dma_start(wgT[i][:], wg_d[i])

            # head: h[jt-block, g-tokens] = Mhead_t[:, jt].T @ x_aug[g-tokens]
            with nc.named_scope("head"):
                for g in range(TLOC):
                    mh = mh_tiles[g // 4]
                    mo = (g % 4) * D
                    gs, ge = g * B, (g + 1) * B
                    ps = ps1.tile([128, 512], f32)
                    for jt in range(NB):
                        nc.tensor.matmul(ps[:, jt * 64:(jt + 1) * 64],
                                         mh[:, mo + jt * 128:mo + (jt + 1) * 128],
                                         xt[:, gs:ge], start=True, stop=True)
                    psR = ps[:].rearrange("p (n t) -> p n t", n=NB)
                    nc.vector.tensor_copy(hAR[:, :, gs:ge], psR)

            def big_stage(src, dst, w_dram):
                w = wpool.tile([128, NB * D], bf16, tag="bigw")
                for q in range(4):
                    nc.sync.dma_start(w[:, q * 2048:(q + 1) * 2048],
                                      w_dram[:, q * 2048:(q + 1) * 2048])
                for ch in range(NCH):
                    for jt in range(NB):
                        ps = ps1.tile([128, 512], f32)
                        for kt in range(NB):
                            nc.tensor.matmul(
                                ps[:],
                                w[:, kt * D + jt * 128:kt * D + (jt + 1) * 128],
                                src[:, kt * NTOK + ch * 512:kt * NTOK + (ch + 1) * 512],
                                start=(kt == 0), stop=(kt == NB - 1),
                            )
                        nc.scalar.activation(
                            dst[:, jt * NTOK + ch * 512:jt * NTOK + (ch + 1) * 512],
                            ps[:], ACT_COPY)

            def coupling(buf, fg, in_half, out_half):
                # buf[out_half] += tanh(W.T @ buf[in_half]), per 128-block
                for ch in range(NCH):
                    for jt in range(NBH):
                        ps = ps1.tile([128, 512], f32)
                        for kt in range(NBH):
                            nc.tensor.matmul(
                                ps[:],
                                fg[:, kt * HB + jt * 128:kt * HB + (jt + 1) * 128],
                                buf[:, (in_half * NBH + kt) * NTOK + ch * 512:
                                    (in_half * NBH + kt) * NTOK + (ch + 1) * 512],
                                start=(kt == 0), stop=(kt == NBH - 1),
                            )
                        tmp = tpool.tile([128, 512], bf16)
                        nc.scalar.activation(tmp[:], ps[:], ACT_TANH)
                        dsl = buf[:, (out_half * NBH + jt) * NTOK + ch * 512:
                                  (out_half * NBH + jt) * NTOK + (ch + 1) * 512]
                        nc.vector.tensor_add(dsl, dsl, tmp[:])

            def rev_block(buf, i):
                coupling(buf, wfT[i][:], in_half=1, out_half=0)  # y1 = h1 + tanh(F.T h2)
                coupling(buf, wgT[i][:], in_half=0, out_half=1)  # y2 = h2 + tanh(G.T y1)

            def scoped(name, fn, *args, **kw):
                with nc.named_scope(name):
                    fn(*args, **kw)

            for i in range(3):
                scoped(f"revf{i}", rev_block, hA, i)
            scoped("Srefl", big_stage, hA, hB, ws_d)
            for i in reversed(range(3)):
                scoped(f"revb{i}", rev_block, hB, i)

            # tail: out[g-tokens] = sum_kt Mtail_t[kt].T @ h[kt, g-tokens]
            with nc.named_scope("tail"):
                mt = wpool.tile([128, TLOC * HB], bf16, tag="bigw")
                for q in range(4):
                    nc.sync.dma_start(mt[:, q * 2048:(q + 1) * 2048],
                                      mt_d[:, q * 2048:(q + 1) * 2048])
                for ch in range(NCH):
                    for gl in range(8):
                        g = ch * 8 + gl
                        gs, ge = g * B, (g + 1) * B
                        ps = psw.tile([DOUT, B], f32, tag="sm")
                        for kt in range(NB):
                            nc.tensor.matmul(
                                ps[:],
                                mt[:, g * HB + kt * 64:g * HB + (kt + 1) * 64],
                                hB[:, kt * NTOK + gs:kt * NTOK + ge],
                                start=(kt == 0), stop=(kt == NB - 1))
                        nc.vector.tensor_copy(outsb[:, gs:ge], ps[:])
                    nc.sync.dma_start(out_d[:, ch * 512:(ch + 1) * 512],
                                      outsb[:, ch * 512:(ch + 1) * 512])

    nc.compile()
    return nc


def _host_weights(Wi, bi, P, rotW, F, G, R, Wo):
    """Fold t-independent weights into the SBUF layouts the kernel expects."""
    import ml_dtypes
    bf16 = ml_dtypes.bfloat16
    W12 = rotW[1] @ rotW[2]
    Srefl = R + R.T
    ws = Srefl.reshape(NB, 128, D).transpose(1, 0, 2).reshape(128, NB * D)
    ws = np.ascontiguousarray(ws).astype(bf16)

    wf = np.stack([f.reshape(NBH, 128, HB).transpose(1, 0, 2).reshape(128, NBH * HB)
                   for f in F]).astype(bf16)
    wg = np.stack([g.reshape(NBH, 128, HB).transpose(1, 0, 2).reshape(128, NBH * HB)
                   for g in G]).astype(bf16)

    WpreA = np.concatenate([P @ Wi, (P @ bi)[:, None]], axis=1)  # [D, DIN+1]
    Wpost = P @ Wo.T                                             # [D, DOUT]
    return W12, WpreA, Wpost, ws, wf, wg


def _per_core_mats(c, rotW0, W12, WpreA, Wpost):
    """Per-t folded head/tail matrices for core c, in SBUF layout."""
    import ml_dtypes
    bf16 = ml_dtypes.bfloat16
    ts = [c * TLOC + g for g in range(TLOC)]
    A = np.stack([np.roll(rotW0, (t, t), axis=(0, 1)) for t in ts])  # [16,D,D]
    # Mhead_t = WpreA.T @ A_t @ W12  -> [16, 65, D]
    Mhead = np.matmul(np.matmul(WpreA.T[None], A), W12)
    # Mtail_t = W12.T @ A_t.T @ Wpost -> [16, D, 64]
    Mtail = np.matmul(W12.T[None], np.matmul(A.transpose(0, 2, 1), Wpost))

    # mhead sbuf: [65, g*D + jt*128 + m] = Mhead[g, :, jt*128+m]
    mh = np.ascontiguousarray(
        Mhead.transpose(1, 0, 2).reshape(DIN + 1, TLOC * D)).astype(bf16)
    # mtail sbuf: [p, g*HB + kt*64 + m] = Mtail[g, kt*128+p, m]
    mt = np.ascontiguousarray(
        Mtail.reshape(TLOC, NB, 128, DOUT).transpose(2, 0, 1, 3)
        .reshape(128, TLOC * NB * DOUT)).astype(bf16)
    return mh, mt


def kernel(x, Wi, bi, P, rotW, F, G, R, Wo, bo):
    import ml_dtypes
    bf16 = ml_dtypes.bfloat16
    x = np.asarray(x, np.float32)
    Wi, bi, P = (np.asarray(a, np.float32) for a in (Wi, bi, P))
    rotW, F, G = (np.asarray(a, np.float32) for a in (rotW, F, G))
    R, Wo, bo = (np.asarray(a, np.float32) for a in (R, Wo, bo))

    if "nc" not in _compiled:
        _compiled["nc"] = _build()
    nc = _compiled["nc"]

    W12, WpreA, Wpost, ws, wf, wg = _host_weights(Wi, bi, P, rotW, F, G, R, Wo)

    in_maps = []
    for c in range(NCORES):
        # xt[din, g*B + b] = x[b, c*TLOC + g, din]; ones row for the bias
        xs = x[:, c * TLOC:(c + 1) * TLOC, :]          # [B, TLOC, DIN]
        xT = xs.transpose(2, 1, 0).reshape(DIN, NTOK)  # [DIN, g*B+b]
        xT = np.concatenate([xT, np.ones((1, NTOK), np.float32)], axis=0)
        mh, mt = _per_core_mats(c, rotW[0], W12, WpreA, Wpost)
        in_maps.append({
            "xt": np.ascontiguousarray(xT).astype(bf16),
            "mhead": mh, "mtail": mt,
            "wf": wf, "wg": wg, "wsrefl": ws,
        })

    from concourse.bass_utils import run_bass_kernel_spmd
    res = run_bass_kernel_spmd(nc, in_maps, list(range(NCORES)))
    _compiled["last_res"] = res

    out = np.empty((B, S, DOUT), np.float32)
    for c in range(NCORES):
        oT = res.results[c]["out"]                     # [DOUT, NTOK]
        out[:, c * TLOC:(c + 1) * TLOC, :] = \
            oT.reshape(DOUT, TLOC, B).transpose(2, 1, 0)
    out += bo.astype(np.float32)
    return out


# revision 10
# speedup vs baseline: 1.1236x; 1.1236x over previous
"""Trainium2 Bass kernel for the Enigma-style CopyMemoryModel.

Math (validated vs reference, see check_fold.py):
  - The lax.scan carries nothing -> every timestep t is independent.
  - t < 128 and d = 1024  =>  rotors 1,2 have pos = 0 (no roll); only rotor 0
    rolls by t, and roll(roll(h,-t) @ W, t) == h @ roll(W, (t,t), (0,1)).
  - Everything before the first rev block is LINEAR with the only t-dependence
    being rotor 0's roll -> fold on host into per-t head matrices
        Mhead_t = [P@Wi | P@bi].T @ roll(rotW0,(t,t)) @ (rotW1@rotW2)   [65,1024]
  - Everything after the last rev block is linear too -> per-t tail matrices
        Mtail_t = (rotW1@rotW2).T @ roll(rotW0,(t,t)).T @ (P@Wo.T)      [1024,64]
  - On chip only:  head (per-t small matmuls), 6 rev couplings fwd,
    Srefl = R+R.T big stage, 6 rev couplings bwd, tail.  bo added on host.
  - Layout on chip: activations stored transposed, hT[128 part, 8 blocks x 1024
    tokens] per core; every stage is out_block[jt] = sum_kt W[kt,jt].T @ h[kt].
  - bf16 datapath (PSUM accumulation fp32, final output fp32): bf16 weights
    enable background LDWEIGHTS + fast-weight-load, so N=512 matmuls stream at
    ~216 ns instead of fp32r's serialized ~355 ns.

Sharding: time-sharded; core c handles t in [c*16, (c+1)*16), all 64 batch
samples -> 1024 tokens per core, token column = g*64 + b.
"""
import numpy as np

B, S, DIN, D, DOUT = 64, 128, 64, 1024, 64
NCORES = 8
TLOC = S // NCORES          # 16 timesteps per core
NTOK = B * TLOC             # 1024 tokens per core
NB = D // 128               # 8 row blocks
NCH = NTOK // 512           # 2 column chunks of 512
HB = 512                    # half of D (rev-block split)
NBH = HB // 128             # 4 blocks per half

_compiled = {}


def _build():
    import concourse.bacc as bacc
    import concourse.mybir as mybir
    from concourse.tile import TileContext

    f32 = mybir.dt.float32
    bf16 = mybir.dt.bfloat16
    ACT_TANH = mybir.ActivationFunctionType.Tanh
    ACT_COPY = mybir.ActivationFunctionType.Copy

    nc = bacc.Bacc(None, target_bir_lowering=False, debug=True)

    xt_d = nc.dram_tensor("xt", [DIN + 1, NTOK], bf16, kind="ExternalInput")
    mh_d = nc.dram_tensor("mhead", [DIN + 1, TLOC * D], bf16, kind="ExternalInput")
    wf_d = nc.dram_tensor("wf", [3, 128, NBH * HB], bf16, kind="ExternalInput")
    wg_d = nc.dram_tensor("wg", [3, 128, NBH * HB], bf16, kind="ExternalInput")
    ws_d = nc.dram_tensor("wsrefl", [128, NB * D], bf16, kind="ExternalInput")
    mt_d = nc.dram_tensor("mtail", [128, TLOC * HB], bf16, kind="ExternalInput")
    out_d = nc.dram_tensor("out", [DOUT, NTOK], f32, kind="ExternalOutput")

    with TileContext(nc) as tc:
        with (
            tc.tile_pool(name="hbuf", bufs=1) as hpool,
            tc.tile_pool(name="wpool", bufs=2) as wpool,
            tc.tile_pool(name="fgpool", bufs=1) as fgpool,
            tc.tile_pool(name="hdpool", bufs=1) as hdpool,
            tc.tile_pool(name="cpool", bufs=1) as cpool,
            tc.tile_pool(name="tpool", bufs=2) as tpool,
            tc.tile_pool(name="ps1", bufs=6, space="PSUM") as ps1,
            tc.tile_pool(name="psw", bufs=2, space="PSUM") as psw,
        ):
            hA = hpool.tile([128, NB * NTOK], bf16)
            hB = hpool.tile([128, NB * NTOK], bf16)
            hAR = hA[:].rearrange("p (n t) -> p n t", n=NB)

            xt = cpool.tile([DIN + 1, NTOK], bf16)
            outsb = cpool.tile([DOUT, NTOK], f32)

            # PE-warmup matmuls on a memset tile queue first so the tensor
            # engine has work while the input DMAs land.
            junk = cpool.tile([128, 512], bf16)
            nc.gpsimd.memset(junk[:], 0.0)

            def junk_mm(n):
                for r in range(n):
                    wps = psw.tile([128, 512], f32, tag="sm")
                    nc.tensor.matmul(wps[:, 0:128], junk[:, 0:128],
                                     junk[:, 0:128], start=True, stop=True)

            with nc.named_scope("warmup"):
                junk_mm(12)

            # first-wave DMAs spread across engine queues: dma_start costs
            # ~1us serial issue per instruction, so sync alone starves the head
            nc.sync.dma_start(xt[:], xt_d[:])
            mh_tiles = []
            for q in range(4):
                mh = hdpool.tile([DIN + 1, 4 * D], bf16, tag=f"mh{q}",
                                 name=f"mh{q}")
                nc.gpsimd.dma_start(mh[:], mh_d[:, q * 4 * D:(q + 1) * 4 * D])
                mh_tiles.append(mh)

            wfT = [fgpool.tile([128, NBH * HB], bf16, tag=f"wf{i}",
                               name=f"wf{i}") for i in range(3)]
            wgT = [fgpool.tile([128, NBH * HB], bf16, tag=f"wg{i}",
                               name=f"wg{i}") for i in range(3)]
            nc.sync.dma_start(wfT[0][:], wf_d[0])
            nc.sync.dma_start(wgT[0][:], wg_d[0])
            for i in range(1, 3):
                nc.sync.dma_start(wfT[i][:], wf_d[i])
                nc.sync.dma_start(wgT[i][:], wg_d[i])

            # head: h[jt-block, g-tokens] = Mhead_t[:, jt].T @ x_aug[g-tokens]
            def head_group(g):
                mh = mh_tiles[g // 4]
                mo = (g % 4) * D
                gs, ge = g * B, (g + 1) * B
                ps = ps1.tile([128, 512], f32)
                for jt in range(NB):
                    nc.tensor.matmul(ps[:, jt * 64:(jt + 1) * 64],
                                     mh[:, mo + jt * 128:mo + (jt + 1) * 128],
                                     xt[:, gs:ge], start=True, stop=True)
                psR = ps[:].rearrange("p (n t) -> p n t", n=NB)
                nc.vector.tensor_copy(hAR[:, :, gs:ge], psR)

            def big_stage(src, dst, w_dram):
                w = wpool.tile([128, NB * D], bf16, tag="bigw")
                for q in range(4):
                    nc.sync.dma_start(w[:, q * 2048:(q + 1) * 2048],
                                      w_dram[:, q * 2048:(q + 1) * 2048])
                for ch in range(NCH):
                    for jt in range(NB):
                        ps = ps1.tile([128, 512], f32)
                        for kt in range(NB):
                            nc.tensor.matmul(
                                ps[:],
                                w[:, kt * D + jt * 128:kt * D + (jt + 1) * 128],
                                src[:, kt * NTOK + ch * 512:kt * NTOK + (ch + 1) * 512],
                                start=(kt == 0), stop=(kt == NB - 1),
                            )
                        nc.scalar.activation(
                            dst[:, jt * NTOK + ch * 512:jt * NTOK + (ch + 1) * 512],
                            ps[:], ACT_COPY)

            def coupling_chunk(buf, fg, in_half, out_half, ch):
                # buf[out_half] += tanh(W.T @ buf[in_half]) for token chunk ch
                for jt in range(NBH):
                    ps = ps1.tile([128, 512], f32)
                    for kt in range(NBH):
                        nc.tensor.matmul(
                            ps[:],
                            fg[:, kt * HB + jt * 128:kt * HB + (jt + 1) * 128],
                            buf[:, (in_half * NBH + kt) * NTOK + ch * 512:
                                (in_half * NBH + kt) * NTOK + (ch + 1) * 512],
                            start=(kt == 0), stop=(kt == NBH - 1),
                        )
                    tmp = tpool.tile([128, 512], bf16)
                    nc.scalar.activation(tmp[:], ps[:], ACT_TANH)
                    dsl = buf[:, (out_half * NBH + jt) * NTOK + ch * 512:
                              (out_half * NBH + jt) * NTOK + (ch + 1) * 512]
                    nc.vector.tensor_add(dsl, dsl, tmp[:])

            def coupling(buf, fg, in_half, out_half):
                for ch in range(NCH):
                    coupling_chunk(buf, fg, in_half, out_half, ch)

            def rev_block(buf, i):
                coupling(buf, wfT[i][:], in_half=1, out_half=0)  # y1 = h1 + tanh(F.T h2)
                coupling(buf, wgT[i][:], in_half=0, out_half=1)  # y2 = h2 + tanh(G.T y1)

            def scoped(name, fn, *args, **kw):
                with nc.named_scope(name):
                    fn(*args, **kw)

            # head interleaved with revf0's F-coupling: the coupling chunks
            # give the PE dense work to fill head DMA-arrival gaps
            with nc.named_scope("head"):
                for g in range(8):
                    head_group(g)
            scoped("revf0a", coupling_chunk, hA, wfT[0][:], 1, 0, 0)
            with nc.named_scope("head2"):
                for g in range(8, TLOC):
                    head_group(g)
            scoped("revf0b", coupling_chunk, hA, wfT[0][:], 1, 0, 1)
            scoped("revf0c", coupling, hA, wgT[0][:], 0, 1)
            for i in range(1, 3):
                scoped(f"revf{i}", rev_block, hA, i)
            scoped("Srefl", big_stage, hA, hB, ws_d)
            for i in reversed(range(3)):
                scoped(f"revb{i}", rev_block, hB, i)

            # tail: out[g-tokens] = sum_kt Mtail_t[kt].T @ h[kt, g-tokens]
            with nc.named_scope("tail"):
                mt = wpool.tile([128, TLOC * HB], bf16, tag="bigw")
                for q in range(4):
                    nc.sync.dma_start(mt[:, q * 2048:(q + 1) * 2048],
                                      mt_d[:, q * 2048:(q + 1) * 2048])
                for ch in range(NCH):
                    for gl in range(8):
                        g = ch * 8 + gl
                        gs, ge = g * B, (g + 1) * B
                        ps = psw.tile([DOUT, B], f32, tag="sm")
                        for kt in range(NB):
                            nc.tensor.matmul(
                                ps[:],
                                mt[:, g * HB + kt * 64:g * HB + (kt + 1) * 64],
                                hB[:, kt * NTOK + gs:kt * NTOK + ge],
                                start=(kt == 0), stop=(kt == NB - 1))
                        nc.vector.tensor_copy(outsb[:, gs:ge], ps[:])
                    nc.sync.dma_start(out_d[:, ch * 512:(ch + 1) * 512],
                                      outsb[:, ch * 512:(ch + 1) * 512])

    nc.compile()
    return nc


def _host_weights(Wi, bi, P, rotW, F, G, R, Wo):
    """Fold t-independent weights into the SBUF layouts the kernel expects."""
    import ml_dtypes
    bf16 = ml_dtypes.bfloat16
    W12 = rotW[1] @ rotW[2]
    Srefl = R + R.T
    ws = Srefl.reshape(NB, 128, D).transpose(1, 0, 2).reshape(128, NB * D)
    ws = np.ascontiguousarray(ws).astype(bf16)

    wf = np.stack([f.reshape(NBH, 128, HB).transpose(1, 0, 2).reshape(128, NBH * HB)
                   for f in F]).astype(bf16)
    wg = np.stack([g.reshape(NBH, 128, HB).transpose(1, 0, 2).reshape(128, NBH * HB)
                   for g in G]).astype(bf16)

    WpreA = np.concatenate([P @ Wi, (P @ bi)[:, None]], axis=1)  # [D, DIN+1]
    Wpost = P @ Wo.T                                             # [D, DOUT]
    return W12, WpreA, Wpost, ws, wf, wg


def _per_core_mats(c, rotW0, W12, WpreA, Wpost):
    """Per-t folded head/tail matrices for core c, in SBUF layout."""
    import ml_dtypes
    bf16 = ml_dtypes.bfloat16
    ts = [c * TLOC + g for g in range(TLOC)]
    A = np.stack([np.roll(rotW0, (t, t), axis=(0, 1)) for t in ts])  # [16,D,D]
    # Mhead_t = WpreA.T @ A_t @ W12  -> [16, 65, D]
    Mhead = np.matmul(np.matmul(WpreA.T[None], A), W12)
    # Mtail_t = W12.T @ A_t.T @ Wpost -> [16, D, 64]
    Mtail = np.matmul(W12.T[None], np.matmul(A.transpose(0, 2, 1), Wpost))

    # mhead sbuf: [65, g*D + jt*128 + m] = Mhead[g, :, jt*128+m]
    mh = np.ascontiguousarray(
        Mhead.transpose(1, 0, 2).reshape(DIN + 1, TLOC * D)).astype(bf16)
    # mtail sbuf: [p, g*HB + kt*64 + m] = Mtail[g, kt*128+p, m]
    mt = np.ascontiguousarray(
        Mtail.reshape(TLOC, NB, 128, DOUT).transpose(2, 0, 1, 3)
        .reshape(128, TLOC * NB * DOUT)).astype(bf16)
    return mh, mt


def kernel(x, Wi, bi, P, rotW, F, G, R, Wo, bo):
    import ml_dtypes
    bf16 = ml_dtypes.bfloat16
    x = np.asarray(x, np.float32)
    Wi, bi, P = (np.asarray(a, np.float32) for a in (Wi, bi, P))
    rotW, F, G = (np.asarray(a, np.float32) for a in (rotW, F, G))
    R, Wo, bo = (np.asarray(a, np.float32) for a in (R, Wo, bo))

    if "nc" not in _compiled:
        _compiled["nc"] = _build()
    nc = _compiled["nc"]

    W12, WpreA, Wpost, ws, wf, wg = _host_weights(Wi, bi, P, rotW, F, G, R, Wo)

    in_maps = []
    for c in range(NCORES):
        # xt[din, g*B + b] = x[b, c*TLOC + g, din]; ones row for the bias
        xs = x[:, c * TLOC:(c + 1) * TLOC, :]          # [B, TLOC, DIN]
        xT = xs.transpose(2, 1, 0).reshape(DIN, NTOK)  # [DIN, g*B+b]
        xT = np.concatenate([xT, np.ones((1, NTOK), np.float32)], axis=0)
        mh, mt = _per_core_mats(c, rotW[0], W12, WpreA, Wpost)
        in_maps.append({
            "xt": np.ascontiguousarray(xT).astype(bf16),
            "mhead": mh, "mtail": mt,
            "wf": wf, "wg": wg, "wsrefl": ws,
        })

    from concourse.bass_utils import run_bass_kernel_spmd
    res = run_bass_kernel_spmd(nc, in_maps, list(range(NCORES)))
    _compiled["last_res"] = res

    out = np.empty((B, S, DOUT), np.float32)
    for c in range(NCORES):
        oT = res.results[c]["out"]                     # [DOUT, NTOK]
        out[:, c * TLOC:(c + 1) * TLOC, :] = \
            oT.reshape(DOUT, TLOC, B).transpose(2, 1, 0)
    out += bo.astype(np.float32)
    return out


# revision 12
# speedup vs baseline: 1.1289x; 1.0047x over previous
"""Trainium2 Bass kernel for the Enigma-style CopyMemoryModel.

Math (validated vs reference, see check_fold.py):
  - The lax.scan carries nothing -> every timestep t is independent.
  - t < 128 and d = 1024  =>  rotors 1,2 have pos = 0 (no roll); only rotor 0
    rolls by t, and roll(roll(h,-t) @ W, t) == h @ roll(W, (t,t), (0,1)).
  - Everything before the first rev block is LINEAR with the only t-dependence
    being rotor 0's roll -> fold on host into per-t head matrices
        Mhead_t = [P@Wi | P@bi].T @ roll(rotW0,(t,t)) @ (rotW1@rotW2)   [65,1024]
  - Everything after the last rev block is linear too -> per-t tail matrices
        Mtail_t = (rotW1@rotW2).T @ roll(rotW0,(t,t)).T @ (P@Wo.T)      [1024,64]
  - On chip only:  head (per-t small matmuls), 6 rev couplings fwd,
    Srefl = R+R.T big stage, 6 rev couplings bwd, tail.  bo added on host.
  - Layout on chip: activations stored transposed, hT[128 part, 8 blocks x 1024
    tokens] per core; every stage is out_block[jt] = sum_kt W[kt,jt].T @ h[kt].
  - bf16 datapath (PSUM accumulation fp32, final output fp32): bf16 weights
    enable background LDWEIGHTS + fast-weight-load, so N=512 matmuls stream at
    ~216 ns instead of fp32r's serialized ~355 ns.

Sharding: time-sharded; core c handles t in [c*16, (c+1)*16), all 64 batch
samples -> 1024 tokens per core, token column = g*64 + b.
"""
import numpy as np

B, S, DIN, D, DOUT = 64, 128, 64, 1024, 64
NCORES = 8
TLOC = S // NCORES          # 16 timesteps per core
NTOK = B * TLOC             # 1024 tokens per core
NB = D // 128               # 8 row blocks
NCH = NTOK // 512           # 2 column chunks of 512
HB = 512                    # half of D (rev-block split)
NBH = HB // 128             # 4 blocks per half

_compiled = {}


def _build():
    import concourse.bacc as bacc
    import concourse.mybir as mybir
    from concourse.tile import TileContext

    f32 = mybir.dt.float32
    bf16 = mybir.dt.bfloat16
    ACT_TANH = mybir.ActivationFunctionType.Tanh
    ACT_COPY = mybir.ActivationFunctionType.Copy

    nc = bacc.Bacc(None, target_bir_lowering=False, debug=True)

    xt_d = nc.dram_tensor("xt", [DIN + 1, NTOK], bf16, kind="ExternalInput")
    mh_d = nc.dram_tensor("mhead", [DIN + 1, TLOC * D], bf16, kind="ExternalInput")
    wf_d = nc.dram_tensor("wf", [3, 128, NBH * HB], bf16, kind="ExternalInput")
    wg_d = nc.dram_tensor("wg", [3, 128, NBH * HB], bf16, kind="ExternalInput")
    ws_d = nc.dram_tensor("wsrefl", [128, NB * D], bf16, kind="ExternalInput")
    mt_d = nc.dram_tensor("mtail", [128, TLOC * HB], bf16, kind="ExternalInput")
    out_d = nc.dram_tensor("out", [DOUT, NTOK], f32, kind="ExternalOutput")

    with TileContext(nc) as tc:
        with (
            tc.tile_pool(name="hbuf", bufs=1) as hpool,
            tc.tile_pool(name="wpool", bufs=2) as wpool,
            tc.tile_pool(name="fgpool", bufs=1) as fgpool,
            tc.tile_pool(name="hdpool", bufs=1) as hdpool,
            tc.tile_pool(name="cpool", bufs=1) as cpool,
            tc.tile_pool(name="tpool", bufs=2) as tpool,
            tc.tile_pool(name="ps1", bufs=6, space="PSUM") as ps1,
            tc.tile_pool(name="psw", bufs=2, space="PSUM") as psw,
        ):
            hA = hpool.tile([128, NB * NTOK], bf16)
            hB = hpool.tile([128, NB * NTOK], bf16)
            hAR = hA[:].rearrange("p (n t) -> p n t", n=NB)

            xt = cpool.tile([DIN + 1, NTOK], bf16)
            outsb = cpool.tile([DOUT, NTOK], f32)

            # PE-warmup matmuls on a memset tile queue first so the tensor
            # engine has work while the input DMAs land.
            junk = cpool.tile([128, 512], bf16)
            nc.gpsimd.memset(junk[:], 0.0)

            def junk_mm(n):
                for r in range(n):
                    wps = psw.tile([128, 512], f32, tag="sm")
                    nc.tensor.matmul(wps[:, 0:128], junk[:, 0:128],
                                     junk[:, 0:128], start=True, stop=True)

            with nc.named_scope("warmup"):
                junk_mm(12)

            # first-wave DMAs spread across engine queues: dma_start costs
            # ~1us serial issue per instruction, so one queue alone starves
            # the head.  One mh batch per engine, xt on the lightest queue.
            mh_tiles = []
            for q in range(4):
                mh = hdpool.tile([DIN + 1, 4 * D], bf16, tag=f"mh{q}",
                                 name=f"mh{q}")
                mh_tiles.append(mh)
            nc.sync.dma_start(mh_tiles[0][:], mh_d[:, 0:4 * D])
            nc.scalar.dma_start(xt[:], xt_d[:])
            nc.scalar.dma_start(mh_tiles[1][:], mh_d[:, 4 * D:8 * D])
            nc.gpsimd.dma_start(mh_tiles[2][:], mh_d[:, 8 * D:12 * D])
            nc.scalar.dma_start(mh_tiles[3][:], mh_d[:, 12 * D:16 * D])

            wfT = [fgpool.tile([128, NBH * HB], bf16, tag=f"wf{i}",
                               name=f"wf{i}") for i in range(3)]
            wgT = [fgpool.tile([128, NBH * HB], bf16, tag=f"wg{i}",
                               name=f"wg{i}") for i in range(3)]
            nc.sync.dma_start(wfT[0][:], wf_d[0])
            nc.sync.dma_start(wgT[0][:], wg_d[0])
            for i in range(1, 3):
                nc.sync.dma_start(wfT[i][:], wf_d[i])
                nc.sync.dma_start(wgT[i][:], wg_d[i])

            # head: h[jt-block, g-tokens] = Mhead_t[:, jt].T @ x_aug[g-tokens]
            def head_group(g):
                mh = mh_tiles[g // 4]
                mo = (g % 4) * D
                gs, ge = g * B, (g + 1) * B
                ps = ps1.tile([128, 512], f32)
                for jt in range(NB):
                    nc.tensor.matmul(ps[:, jt * 64:(jt + 1) * 64],
                                     mh[:, mo + jt * 128:mo + (jt + 1) * 128],
                                     xt[:, gs:ge], start=True, stop=True)
                psR = ps[:].rearrange("p (n t) -> p n t", n=NB)
                nc.vector.tensor_copy(hAR[:, :, gs:ge], psR)

            def big_stage(src, dst, w_dram):
                w = wpool.tile([128, NB * D], bf16, tag="bigw")
                for q in range(4):
                    nc.sync.dma_start(w[:, q * 2048:(q + 1) * 2048],
                                      w_dram[:, q * 2048:(q + 1) * 2048])
                for ch in range(NCH):
                    for jt in range(NB):
                        ps = ps1.tile([128, 512], f32)
                        for kt in range(NB):
                            nc.tensor.matmul(
                                ps[:],
                                w[:, kt * D + jt * 128:kt * D + (jt + 1) * 128],
                                src[:, kt * NTOK + ch * 512:kt * NTOK + (ch + 1) * 512],
                                start=(kt == 0), stop=(kt == NB - 1),
                            )
                        nc.scalar.activation(
                            dst[:, jt * NTOK + ch * 512:jt * NTOK + (ch + 1) * 512],
                            ps[:], ACT_COPY)

            def coupling_chunk(buf, fg, in_half, out_half, ch):
                # buf[out_half] += tanh(W.T @ buf[in_half]) for token chunk ch
                for jt in range(NBH):
                    ps = ps1.tile([128, 512], f32)
                    for kt in range(NBH):
                        nc.tensor.matmul(
                            ps[:],
                            fg[:, kt * HB + jt * 128:kt * HB + (jt + 1) * 128],
                            buf[:, (in_half * NBH + kt) * NTOK + ch * 512:
                                (in_half * NBH + kt) * NTOK + (ch + 1) * 512],
                            start=(kt == 0), stop=(kt == NBH - 1),
                        )
                    tmp = tpool.tile([128, 512], bf16)
                    nc.scalar.activation(tmp[:], ps[:], ACT_TANH)
                    dsl = buf[:, (out_half * NBH + jt) * NTOK + ch * 512:
                              (out_half * NBH + jt) * NTOK + (ch + 1) * 512]
                    nc.vector.tensor_add(dsl, dsl, tmp[:])

            def coupling(buf, fg, in_half, out_half):
                for ch in range(NCH):
                    coupling_chunk(buf, fg, in_half, out_half, ch)

            def rev_block(buf, i):
                coupling(buf, wfT[i][:], in_half=1, out_half=0)  # y1 = h1 + tanh(F.T h2)
                coupling(buf, wgT[i][:], in_half=0, out_half=1)  # y2 = h2 + tanh(G.T y1)

            def scoped(name, fn, *args, **kw):
                with nc.named_scope(name):
                    fn(*args, **kw)

            # head interleaved with revf0's F-coupling: the coupling chunks
            # give the PE dense work to fill head DMA-arrival gaps
            with nc.named_scope("head"):
                for g in range(8):
                    head_group(g)
            scoped("revf0a", coupling_chunk, hA, wfT[0][:], 1, 0, 0)
            with nc.named_scope("head2"):
                for g in range(8, TLOC):
                    head_group(g)
            scoped("revf0b", coupling_chunk, hA, wfT[0][:], 1, 0, 1)
            scoped("revf0c", coupling, hA, wgT[0][:], 0, 1)
            for i in range(1, 3):
                scoped(f"revf{i}", rev_block, hA, i)
            scoped("Srefl", big_stage, hA, hB, ws_d)
            for i in reversed(range(3)):
                scoped(f"revb{i}", rev_block, hB, i)

            # tail: out[g-tokens] = sum_kt Mtail_t[kt].T @ h[kt, g-tokens]
            with nc.named_scope("tail"):
                mt = wpool.tile([128, TLOC * HB], bf16, tag="bigw")
                for q in range(4):
                    nc.sync.dma_start(mt[:, q * 2048:(q + 1) * 2048],
                                      mt_d[:, q * 2048:(q + 1) * 2048])
                for ch in range(NCH):
                    for gl in range(8):
                        g = ch * 8 + gl
                        gs, ge = g * B, (g + 1) * B
                        ps = psw.tile([DOUT, B], f32, tag="sm")
                        for kt in range(NB):
                            nc.tensor.matmul(
                                ps[:],
                                mt[:, g * HB + kt * 64:g * HB + (kt + 1) * 64],
                                hB[:, kt * NTOK + gs:kt * NTOK + ge],
                                start=(kt == 0), stop=(kt == NB - 1))
                        nc.vector.tensor_copy(outsb[:, gs:ge], ps[:])
                    nc.sync.dma_start(out_d[:, ch * 512:(ch + 1) * 512],
                                      outsb[:, ch * 512:(ch + 1) * 512])

    nc.compile()
    return nc


def _host_weights(Wi, bi, P, rotW, F, G, R, Wo):
    """Fold t-independent weights into the SBUF layouts the kernel expects."""
    import ml_dtypes
    bf16 = ml_dtypes.bfloat16
    W12 = rotW[1] @ rotW[2]
    Srefl = R + R.T
    ws = Srefl.reshape(NB, 128, D).transpose(1, 0, 2).reshape(128, NB * D)
    ws = np.ascontiguousarray(ws).astype(bf16)

    wf = np.stack([f.reshape(NBH, 128, HB).transpose(1, 0, 2).reshape(128, NBH * HB)
                   for f in F]).astype(bf16)
    wg = np.stack([g.reshape(NBH, 128, HB).transpose(1, 0, 2).reshape(128, NBH * HB)
                   for g in G]).astype(bf16)

    WpreA = np.concatenate([P @ Wi, (P @ bi)[:, None]], axis=1)  # [D, DIN+1]
    Wpost = P @ Wo.T                                             # [D, DOUT]
    return W12, WpreA, Wpost, ws, wf, wg


def _per_core_mats(c, rotW0, W12, WpreA, Wpost):
    """Per-t folded head/tail matrices for core c, in SBUF layout."""
    import ml_dtypes
    bf16 = ml_dtypes.bfloat16
    ts = [c * TLOC + g for g in range(TLOC)]
    A = np.stack([np.roll(rotW0, (t, t), axis=(0, 1)) for t in ts])  # [16,D,D]
    # Mhead_t = WpreA.T @ A_t @ W12  -> [16, 65, D]
    Mhead = np.matmul(np.matmul(WpreA.T[None], A), W12)
    # Mtail_t = W12.T @ A_t.T @ Wpost -> [16, D, 64]
    Mtail = np.matmul(W12.T[None], np.matmul(A.transpose(0, 2, 1), Wpost))

    # mhead sbuf: [65, g*D + jt*128 + m] = Mhead[g, :, jt*128+m]
    mh = np.ascontiguousarray(
        Mhead.transpose(1, 0, 2).reshape(DIN + 1, TLOC * D)).astype(bf16)
    # mtail sbuf: [p, g*HB + kt*64 + m] = Mtail[g, kt*128+p, m]
    mt = np.ascontiguousarray(
        Mtail.reshape(TLOC, NB, 128, DOUT).transpose(2, 0, 1, 3)
        .reshape(128, TLOC * NB * DOUT)).astype(bf16)
    return mh, mt


def kernel(x, Wi, bi, P, rotW, F, G, R, Wo, bo):
    import ml_dtypes
    bf16 = ml_dtypes.bfloat16
    x = np.asarray(x, np.float32)
    Wi, bi, P = (np.asarray(a, np.float32) for a in (Wi, bi, P))
    rotW, F, G = (np.asarray(a, np.float32) for a in (rotW, F, G))
    R, Wo, bo = (np.asarray(a, np.float32) for a in (R, Wo, bo))

    if "nc" not in _compiled:
        _compiled["nc"] = _build()
    nc = _compiled["nc"]

    W12, WpreA, Wpost, ws, wf, wg = _host_weights(Wi, bi, P, rotW, F, G, R, Wo)

    in_maps = []
    for c in range(NCORES):
        # xt[din, g*B + b] = x[b, c*TLOC + g, din]; ones row for the bias
        xs = x[:, c * TLOC:(c + 1) * TLOC, :]          # [B, TLOC, DIN]
        xT = xs.transpose(2, 1, 0).reshape(DIN, NTOK)  # [DIN, g*B+b]
        xT = np.concatenate([xT, np.ones((1, NTOK), np.float32)], axis=0)
        mh, mt = _per_core_mats(c, rotW[0], W12, WpreA, Wpost)
        in_maps.append({
            "xt": np.ascontiguousarray(xT).astype(bf16),
            "mhead": mh, "mtail": mt,
            "wf": wf, "wg": wg, "wsrefl": ws,
        })

    from concourse.bass_utils import run_bass_kernel_spmd
    res = run_bass_kernel_spmd(nc, in_maps, list(range(NCORES)))
    _compiled["last_res"] = res

    out = np.empty((B, S, DOUT), np.float32)
    for c in range(NCORES):
        oT = res.results[c]["out"]                     # [DOUT, NTOK]
        out[:, c * TLOC:(c + 1) * TLOC, :] = \
            oT.reshape(DOUT, TLOC, B).transpose(2, 1, 0)
    out += bo.astype(np.float32)
    return out


# revision 18
# speedup vs baseline: 1.1444x; 1.0137x over previous
"""Trainium2 Bass kernel for the Enigma-style CopyMemoryModel.

Math (validated vs reference, see check_fold.py):
  - The lax.scan carries nothing -> every timestep t is independent.
  - t < 128 and d = 1024  =>  rotors 1,2 have pos = 0 (no roll); only rotor 0
    rolls by t, and roll(roll(h,-t) @ W, t) == h @ roll(W, (t,t), (0,1)).
  - Everything before the first rev block is LINEAR with the only t-dependence
    being rotor 0's roll -> fold on host into per-t head matrices
        Mhead_t = [P@Wi | P@bi].T @ roll(rotW0,(t,t)) @ (rotW1@rotW2)   [65,1024]
  - Everything after the last rev block is linear too -> per-t tail matrices
        Mtail_t = (rotW1@rotW2).T @ roll(rotW0,(t,t)).T @ (P@Wo.T)      [1024,64]
  - On chip only:  head (per-t small matmuls), 6 rev couplings fwd,
    Srefl = R+R.T big stage, 6 rev couplings bwd, tail.  bo added on host.
  - Layout on chip: activations stored transposed, hT[128 part, 8 blocks x 1024
    tokens] per core; every stage is out_block[jt] = sum_kt W[kt,jt].T @ h[kt].
  - bf16 datapath (PSUM accumulation fp32, final output fp32): bf16 weights
    enable background LDWEIGHTS + fast-weight-load, so N=512 matmuls stream at
    ~216 ns instead of fp32r's serialized ~355 ns.

Sharding: time-sharded; core c handles t in [c*16, (c+1)*16), all 64 batch
samples -> 1024 tokens per core, token column = g*64 + b.
"""
import numpy as np

B, S, DIN, D, DOUT = 64, 128, 64, 1024, 64
NCORES = 8
TLOC = S // NCORES          # 16 timesteps per core
NTOK = B * TLOC             # 1024 tokens per core
NB = D // 128               # 8 row blocks
NCH = NTOK // 512           # 2 column chunks of 512
HB = 512                    # half of D (rev-block split)
NBH = HB // 128             # 4 blocks per half

_compiled = {}


def _build():
    import concourse.bacc as bacc
    import concourse.mybir as mybir
    from concourse.tile import TileContext

    f32 = mybir.dt.float32
    bf16 = mybir.dt.bfloat16
    ACT_TANH = mybir.ActivationFunctionType.Tanh
    ACT_COPY = mybir.ActivationFunctionType.Copy

    nc = bacc.Bacc(None, target_bir_lowering=False, debug=True)

    xt_d = nc.dram_tensor("xt", [DIN, NTOK], bf16, kind="ExternalInput")
    mh_d = nc.dram_tensor("mhead", [DIN, TLOC * D], bf16, kind="ExternalInput")
    mhb_d = nc.dram_tensor("mheadb", [1, TLOC * D], bf16, kind="ExternalInput")
    wf_d = nc.dram_tensor("wf", [3, 128, NBH * HB], bf16, kind="ExternalInput")
    wg_d = nc.dram_tensor("wg", [3, 128, NBH * HB], bf16, kind="ExternalInput")
    ws_d = nc.dram_tensor("wsrefl", [128, NB * D], bf16, kind="ExternalInput")
    mt_d = nc.dram_tensor("mtail", [128, TLOC * HB], bf16, kind="ExternalInput")
    out_d = nc.dram_tensor("out", [DOUT, NTOK], f32, kind="ExternalOutput")

    with TileContext(nc) as tc:
        with (
            tc.tile_pool(name="hbuf", bufs=1) as hpool,
            tc.tile_pool(name="wpool", bufs=2) as wpool,
            tc.tile_pool(name="fgpool", bufs=1) as fgpool,
            tc.tile_pool(name="hdpool", bufs=1) as hdpool,
            tc.tile_pool(name="cpool", bufs=1) as cpool,
            tc.tile_pool(name="tpool", bufs=2) as tpool,
            tc.tile_pool(name="ps1", bufs=6, space="PSUM") as ps1,
            tc.tile_pool(name="psw", bufs=2, space="PSUM") as psw,
        ):
            hA = hpool.tile([128, NB * NTOK], bf16)
            hB = hpool.tile([128, NB * NTOK], bf16)
            hAR = hA[:].rearrange("p (n t) -> p n t", n=NB)

            xt = cpool.tile([DIN + 1, NTOK], bf16)
            outsb = cpool.tile([DOUT, NTOK], f32)

            # PE-warmup matmuls on a memset tile queue first so the tensor
            # engine has work while the input DMAs land.
            junk = cpool.tile([128, 512], bf16)
            nc.gpsimd.memset(junk[:], 0.0)

            def junk_mm(n):
                for r in range(n):
                    wps = psw.tile([128, 512], f32, tag="sm")
                    nc.tensor.matmul(wps[:, 0:128], junk[:, 0:128],
                                     junk[:, 0:128], start=True, stop=True)

            with nc.named_scope("warmup"):
                junk_mm(12)

            # first-wave DMAs spread across engine queues: dma_start costs
            # ~1us serial issue per instruction, and a >64-partition dst
            # splits into TWO serially-dependent instructions.  Load the
            # 64-row main parts (single-instruction DMAs) in 4 parallel
            # batches; the bias row comes as one tiny strip, and xt's ones
            # row is a memset.
            mhall = hdpool.tile([DIN + 1, TLOC * D], bf16)
            nc.gpsimd.memset(xt[DIN:DIN + 1, :], 1.0)
            nc.sync.dma_start(mhall[0:DIN, 0:4 * D], mh_d[:, 0:4 * D])
            nc.scalar.dma_start(xt[0:DIN, :], xt_d[:])
            nc.sync.dma_start(mhall[DIN:DIN + 1, :], mhb_d[:])
            nc.scalar.dma_start(mhall[0:DIN, 4 * D:8 * D], mh_d[:, 4 * D:8 * D])
            nc.gpsimd.dma_start(mhall[0:DIN, 8 * D:12 * D], mh_d[:, 8 * D:12 * D])
            nc.scalar.dma_start(mhall[0:DIN, 12 * D:16 * D], mh_d[:, 12 * D:16 * D])

            wfT = [fgpool.tile([128, NBH * HB], bf16, tag=f"wf{i}",
                               name=f"wf{i}") for i in range(3)]
            wgT = [fgpool.tile([128, NBH * HB], bf16, tag=f"wg{i}",
                               name=f"wg{i}") for i in range(3)]
            nc.sync.dma_start(wfT[0][:], wf_d[0])
            nc.sync.dma_start(wgT[0][:], wg_d[0])
            for i in range(1, 3):
                nc.sync.dma_start(wfT[i][:], wf_d[i])
                nc.sync.dma_start(wgT[i][:], wg_d[i])

            # head: h[jt-block, g-tokens] = Mhead_t[:, jt].T @ x_aug[g-tokens]
            def head_group(g):
                mh = mhall
                mo = g * D
                gs, ge = g * B, (g + 1) * B
                ps = ps1.tile([128, 512], f32)
                for jt in range(NB):
                    nc.tensor.matmul(ps[:, jt * 64:(jt + 1) * 64],
                                     mh[:, mo + jt * 128:mo + (jt + 1) * 128],
                                     xt[:, gs:ge], start=True, stop=True)
                psR = ps[:].rearrange("p (n t) -> p n t", n=NB)
                nc.vector.tensor_copy(hAR[:, :, gs:ge], psR)

            def big_stage(src, dst, w_dram):
                w = wpool.tile([128, NB * D], bf16, tag="bigw")
                for q in range(4):
                    nc.sync.dma_start(w[:, q * 2048:(q + 1) * 2048],
                                      w_dram[:, q * 2048:(q + 1) * 2048])
                for ch in range(NCH):
                    for jt in range(NB):
                        ps = ps1.tile([128, 512], f32)
                        for kt in range(NB):
                            nc.tensor.matmul(
                                ps[:],
                                w[:, kt * D + jt * 128:kt * D + (jt + 1) * 128],
                                src[:, kt * NTOK + ch * 512:kt * NTOK + (ch + 1) * 512],
                                start=(kt == 0), stop=(kt == NB - 1),
                            )
                        nc.scalar.activation(
                            dst[:, jt * NTOK + ch * 512:jt * NTOK + (ch + 1) * 512],
                            ps[:], ACT_COPY)

            def coupling_chunk(buf, fg, in_half, out_half, ch):
                # buf[out_half] += tanh(W.T @ buf[in_half]) for token chunk ch
                for jt in range(NBH):
                    ps = ps1.tile([128, 512], f32)
                    for kt in range(NBH):
                        nc.tensor.matmul(
                            ps[:],
                            fg[:, kt * HB + jt * 128:kt * HB + (jt + 1) * 128],
                            buf[:, (in_half * NBH + kt) * NTOK + ch * 512:
                                (in_half * NBH + kt) * NTOK + (ch + 1) * 512],
                            start=(kt == 0), stop=(kt == NBH - 1),
                        )
                    tmp = tpool.tile([128, 512], bf16)
                    nc.scalar.activation(tmp[:], ps[:], ACT_TANH)
                    dsl = buf[:, (out_half * NBH + jt) * NTOK + ch * 512:
                              (out_half * NBH + jt) * NTOK + (ch + 1) * 512]
                    nc.vector.tensor_add(dsl, dsl, tmp[:])

            def coupling(buf, fg, in_half, out_half):
                for ch in range(NCH):
                    coupling_chunk(buf, fg, in_half, out_half, ch)

            def rev_block(buf, i):
                coupling(buf, wfT[i][:], in_half=1, out_half=0)  # y1 = h1 + tanh(F.T h2)
                coupling(buf, wgT[i][:], in_half=0, out_half=1)  # y2 = h2 + tanh(G.T y1)

            def scoped(name, fn, *args, **kw):
                with nc.named_scope(name):
                    fn(*args, **kw)

            # head interleaved with revf0's F-coupling: the coupling chunks
            # give the PE dense work to fill head DMA-arrival gaps
            with nc.named_scope("head"):
                for g in range(8):
                    head_group(g)
            scoped("revf0a", coupling_chunk, hA, wfT[0][:], 1, 0, 0)
            with nc.named_scope("head2"):
                for g in range(8, TLOC):
                    head_group(g)
            scoped("revf0b", coupling_chunk, hA, wfT[0][:], 1, 0, 1)
            scoped("revf0c", coupling, hA, wgT[0][:], 0, 1)
            for i in range(1, 3):
                scoped(f"revf{i}", rev_block, hA, i)
            scoped("Srefl", big_stage, hA, hB, ws_d)
            for i in reversed(range(3)):
                scoped(f"revb{i}", rev_block, hB, i)

            # tail: out[g-tokens] = sum_kt Mtail_t[kt].T @ h[kt, g-tokens]
            with nc.named_scope("tail"):
                mt = wpool.tile([128, TLOC * HB], bf16, tag="bigw")
                for q in range(4):
                    nc.sync.dma_start(mt[:, q * 2048:(q + 1) * 2048],
                                      mt_d[:, q * 2048:(q + 1) * 2048])
                for ch in range(NCH):
                    for gl in range(8):
                        g = ch * 8 + gl
                        gs, ge = g * B, (g + 1) * B
                        ps = psw.tile([DOUT, B], f32, tag="sm")
                        for kt in range(NB):
                            nc.tensor.matmul(
                                ps[:],
                                mt[:, g * HB + kt * 64:g * HB + (kt + 1) * 64],
                                hB[:, kt * NTOK + gs:kt * NTOK + ge],
                                start=(kt == 0), stop=(kt == NB - 1))
                        nc.vector.tensor_copy(outsb[:, gs:ge], ps[:])
                    nc.sync.dma_start(out_d[:, ch * 512:(ch + 1) * 512],
                                      outsb[:, ch * 512:(ch + 1) * 512])

    nc.compile()
    return nc


def _host_weights(Wi, bi, P, rotW, F, G, R, Wo):
    """Fold t-independent weights into the SBUF layouts the kernel expects."""
    import ml_dtypes
    bf16 = ml_dtypes.bfloat16
    W12 = rotW[1] @ rotW[2]
    Srefl = R + R.T
    ws = Srefl.reshape(NB, 128, D).transpose(1, 0, 2).reshape(128, NB * D)
    ws = np.ascontiguousarray(ws).astype(bf16)

    wf = np.stack([f.reshape(NBH, 128, HB).transpose(1, 0, 2).reshape(128, NBH * HB)
                   for f in F]).astype(bf16)
    wg = np.stack([g.reshape(NBH, 128, HB).transpose(1, 0, 2).reshape(128, NBH * HB)
                   for g in G]).astype(bf16)

    WpreA = np.concatenate([P @ Wi, (P @ bi)[:, None]], axis=1)  # [D, DIN+1]
    Wpost = P @ Wo.T                                             # [D, DOUT]
    return W12, WpreA, Wpost, ws, wf, wg


def _per_core_mats(c, rotW0, W12, WpreA, Wpost):
    """Per-t folded head/tail matrices for core c, in SBUF layout."""
    import ml_dtypes
    bf16 = ml_dtypes.bfloat16
    ts = [c * TLOC + g for g in range(TLOC)]
    A = np.stack([np.roll(rotW0, (t, t), axis=(0, 1)) for t in ts])  # [16,D,D]
    # Mhead_t = WpreA.T @ A_t @ W12  -> [16, 65, D]
    Mhead = np.matmul(np.matmul(WpreA.T[None], A), W12)
    # Mtail_t = W12.T @ A_t.T @ Wpost -> [16, D, 64]
    Mtail = np.matmul(W12.T[None], np.matmul(A.transpose(0, 2, 1), Wpost))

    # mhead sbuf: [65, g*D + jt*128 + m] = Mhead[g, :, jt*128+m]
    # split: rows 0..63 (x features) + row 64 (bias) for single-inst DMAs
    mhA = np.ascontiguousarray(
        Mhead.transpose(1, 0, 2).reshape(DIN + 1, TLOC * D)).astype(bf16)
    mh, mhb = mhA[:DIN], mhA[DIN:]
    # mtail sbuf: [p, g*HB + kt*64 + m] = Mtail[g, kt*128+p, m]
    mt = np.ascontiguousarray(
        Mtail.reshape(TLOC, NB, 128, DOUT).transpose(2, 0, 1, 3)
        .reshape(128, TLOC * NB * DOUT)).astype(bf16)
    return mh, mhb, mt


def kernel(x, Wi, bi, P, rotW, F, G, R, Wo, bo):
    import ml_dtypes
    bf16 = ml_dtypes.bfloat16
    x = np.asarray(x, np.float32)
    Wi, bi, P = (np.asarray(a, np.float32) for a in (Wi, bi, P))
    rotW, F, G = (np.asarray(a, np.float32) for a in (rotW, F, G))
    R, Wo, bo = (np.asarray(a, np.float32) for a in (R, Wo, bo))

    if "nc" not in _compiled:
        _compiled["nc"] = _build()
    nc = _compiled["nc"]

    W12, WpreA, Wpost, ws, wf, wg = _host_weights(Wi, bi, P, rotW, F, G, R, Wo)

    in_maps = []
    for c in range(NCORES):
        # xt[din, g*B + b] = x[b, c*TLOC + g, din]; ones row added on chip
        xs = x[:, c * TLOC:(c + 1) * TLOC, :]          # [B, TLOC, DIN]
        xT = xs.transpose(2, 1, 0).reshape(DIN, NTOK)  # [DIN, g*B+b]
        mh, mhb, mt = _per_core_mats(c, rotW[0], W12, WpreA, Wpost)
        in_maps.append({
            "xt": np.ascontiguousarray(xT).astype(bf16),
            "mhead": mh, "mheadb": mhb, "mtail": mt,
            "wf": wf, "wg": wg, "wsrefl": ws,
        })

    from concourse.bass_utils import run_bass_kernel_spmd
    res = run_bass_kernel_spmd(nc, in_maps, list(range(NCORES)))
    _compiled["last_res"] = res

    out = np.empty((B, S, DOUT), np.float32)
    for c in range(NCORES):
        oT = res.results[c]["out"]                     # [DOUT, NTOK]
        out[:, c * TLOC:(c + 1) * TLOC, :] = \
            oT.reshape(DOUT, TLOC, B).transpose(2, 1, 0)
    out += bo.astype(np.float32)
    return out
